# revision 4
# baseline (speedup 1.0000x reference)
"""Trainium2 Bass kernel for nn_CrossAttention (B=2, TGT=1024, SRC=2048,
H=1024, 16 heads x 64).

Sharding: 8 cores = 2 (batch) x 4 (head groups of 4 heads). Each core
computes q/k/v projections for its 4 heads (column-sliced weights), the
attention for those heads, and a partial out-projection (row-sliced Wo).
The host sums the 4 partial out-projections per batch and adds bo.

v6 (scheduling rewrite of the v5 baseline; same all-bf16 math):
  * The kernel is PE-bound (~96us of matmul rows at 2.4GHz) with the DMA
    stream (~31.5MB at ~330GB/s) just underneath.  The v5 baseline lost
    ~60us to HAM cold-clock oscillation (PE idle gaps re-throttle the PE
    clock to 1.2GHz), a 15us startup, and a 12us serial tail.
  * PV software pipeline is restructured so each unit's PV drains as
    early as possible: unit u's j0 half self-drains inside its own
    window (groups 3-7 + 2 leftovers), j1 drains in the first 3 groups
    of the next window.  Softmax norms then land 1-3 groups into the
    next window instead of at its end, which unblocks the out-projection
    (interleaved into window 2) and shrinks the tail to 4 PV matmuls +
    norms + outproj_t1.
  * Projections are interleaved at chunk granularity into the attention
    windows (psA copy latency hides under psL/psV matmuls), x inputs are
    whole-chunk SBUF tiles whose DMAs issue far ahead, and the exp'd
    bias stream prefetches a full unit ahead through a 10-deep pool.
  * Preamble projections cycle their PSUM tiles through the (still
    unused) psV slots as well as psA so back-to-back projection chunks
    never stall on the PSUM->SBUF copy.
  * All projection biases fold away exactly as in v5 (bk cancels, bq
    folds into the host-exponentiated bias, Wo@bv+bo added on host).
"""

import numpy as np
from contextlib import ExitStack

import ml_dtypes

import concourse.bass as bass
import concourse.tile as tile
from concourse import bacc, mybir
from concourse.bass_utils import run_bass_kernel_spmd

P = 128
H_DIM = 1024
N_HEADS = 16
HEAD_DIM = 64
B = 2
TGT = 1024
SRC = 2048
N_CORES = 8
HPC = 4  # heads per core
DPC = HPC * HEAD_DIM  # 256 projected dims per core
F32 = mybir.dt.float32
BF16 = mybir.dt.bfloat16
NPBF16 = ml_dtypes.bfloat16

TQ = 512  # t-chunk for attention units
S_TILES = SRC // P  # 16
KT = H_DIM // P  # 8 contraction tiles for projections
DT = DPC // P  # 2 d-tiles per core
NQ = TGT // TQ  # 2 t-chunks
NKC = SRC // TQ  # 4 n-chunks for k proj
VG = 4  # m-tile groups for v proj (4 s-tiles each)
NG = S_TILES // 2  # 8 attention groups per unit

# window -> group -> list of (owner_unit, j, m_lo, m_hi) PV drains
PV_TABLE = {
    1: {0: [(0, 0, 0, 7)], 1: [(0, 0, 8, 15)], 2: [(0, 1, 0, 7)],
        3: [(0, 1, 8, 15)], 4: [(1, 0, 0, 3)], 5: [(1, 0, 4, 7)],
        6: [(1, 0, 8, 11)], 7: [(1, 0, 12, 13)]},
    2: {0: [(1, 0, 14, 15), (1, 1, 0, 5)], 1: [(1, 1, 6, 11)],
        2: [(1, 1, 12, 15)], 3: [(2, 0, 0, 3)], 4: [(2, 0, 4, 7)],
        5: [(2, 0, 8, 9)], 6: [(2, 0, 10, 11)], 7: [(2, 0, 12, 13)]},
    3: {0: [(2, 0, 14, 15), (2, 1, 0, 5)], 1: [(2, 1, 6, 11)],
        2: [(2, 1, 12, 15)], 3: [(3, 0, 0, 3), (3, 1, 0, 1)],
        4: [(3, 0, 4, 7), (3, 1, 2, 5)], 5: [(3, 0, 8, 9), (3, 1, 6, 9)],
        6: [(3, 0, 10, 11), (3, 1, 10, 11)],
        7: [(3, 0, 12, 13), (3, 1, 12, 13)]},
}
# window -> group -> (owner_unit, j) norms (emitted before that group's PVs)
NORM_TABLE = {
    1: {2: [(0, 0)], 4: [(0, 1)]},
    2: {1: [(1, 0)], 3: [(1, 1)]},
    3: {1: [(2, 0)], 3: [(2, 1)]},
}

_prog_cache: dict = {}


def _emit(tc: tile.TileContext, outs, ins):
    nc = tc.nc
    xq, xk, xv, eb, wq, wk, wv, wo = ins
    (outT,) = outs
    Exp = mybir.ActivationFunctionType.Exp
    Copy = mybir.ActivationFunctionType.Copy

    with ExitStack() as ctx:
        const = ctx.enter_context(tc.tile_pool(name="const", bufs=1))
        xqpool = ctx.enter_context(tc.tile_pool(name="xqin", bufs=2))
        xkpool = ctx.enter_context(tc.tile_pool(name="xkin", bufs=2))
        xvpool = ctx.enter_context(tc.tile_pool(name="xvin", bufs=3))
        ebpool = ctx.enter_context(tc.tile_pool(name="ebin", bufs=10))
        pgpool = ctx.enter_context(tc.tile_pool(name="pg", bufs=2))
        pppool = ctx.enter_context(tc.tile_pool(name="pp", bufs=2))
        rcpool = ctx.enter_context(tc.tile_pool(name="rcp", bufs=2))
        outp = ctx.enter_context(tc.tile_pool(name="outsb", bufs=3))
        psA = ctx.enter_context(tc.tile_pool(name="psA", bufs=2, space="PSUM"))
        psL = ctx.enter_context(tc.tile_pool(name="psL", bufs=1, space="PSUM"))
        psV = ctx.enter_context(tc.tile_pool(name="psV", bufs=1, space="PSUM"))

        # ---- persistent SBUF tensors ----
        wq_sb = const.tile([P, KT, DPC], BF16)  # [e_part, e_tile, d]
        wk_sb = const.tile([P, KT, DPC], BF16)
        wv_sb = const.tile([P, KT, DPC], BF16)
        wo_sb = const.tile([P, DT, H_DIM], BF16)  # [hd_part, hd_tile, e_out]
        q_sb = const.tile([P, DT, TGT], BF16)  # qT
        k_sb = const.tile([P, DT, SRC], BF16)  # kT
        # v plus 64 ones-columns, per (s_tile, head): [.., 0:64]=v, [.., 64:128]=1
        v_sb = const.tile([P, S_TILES, HPC, P], BF16)
        attn_sb = const.tile([P, DT, TGT], BF16)  # attnT, normalized

        nc.gpsimd.memset(v_sb[:, :, :, HEAD_DIM:P], 1.0)

        # ---- x chunk tiles (one DMA batch per chunk, deep prefetch) ----
        def x_chunk(pool, x_dram, n, nm):
            xt = pool.tile([P, KT, TQ], BF16, name=nm)
            for k in range(KT):
                nc.sync.dma_start(xt[:, k, :], x_dram[k, n])
            return xt

        # ---- q/k projections: psum[d_tile] += wT_tile.T @ xT_tile ----
        def proj_chunk(xt, w_sb, dst_sb, n, slots):
            pss = [slots() for _ in range(DT)]
            for k in range(KT):
                for m in range(DT):
                    nc.tensor.matmul(
                        pss[m][:],
                        lhsT=w_sb[:, k, m * P:(m + 1) * P],
                        rhs=xt[:, k, :],
                        start=(k == 0),
                        stop=(k == KT - 1),
                    )
            for m in range(DT):
                nc.vector.tensor_copy(
                    dst_sb[:, m, n * TQ:(n + 1) * TQ], pss[m][:])

        # ---- v projection group: 4 s-tiles, all heads ----
        def proj_v_group(xvt, mg, slots):
            for ml in range(VG):
                m = mg * VG + ml
                ps = slots()[:, :DPC]
                for k in range(KT):
                    nc.tensor.matmul(
                        ps,
                        lhsT=xvt[:, k, ml * P:(ml + 1) * P],
                        rhs=wv_sb[:, k, :],
                        start=(k == 0),
                        stop=(k == KT - 1),
                    )
                nc.vector.tensor_copy(
                    v_sb[:, m, :, 0:HEAD_DIM],
                    ps.rearrange("p (h d) -> p h d", d=HEAD_DIM),
                )

        # psum slot cyclers.  Preamble rotates through psA + the not-yet-
        # used psV slots so chunk copies never stall; in-window projs and
        # the out-projection use psA only.
        class Slots:
            def __init__(self, seq):
                self.seq = seq
                self.i = 0

            def __call__(self):
                pool, tag = self.seq[self.i % len(self.seq)]
                self.i += 1
                t = pool.tile([P, TQ], F32, name=f"ps_{tag}", tag=tag)
                return t

        pre_slots = Slots([(psA, "mm"), (psV, "pv0"), (psV, "pv1"), (psA, "mm")])
        mm_slots = Slots([(psA, "mm")])

        # ---- attention machinery ----
        units = [(0, 0), (1, 0), (0, 1), (1, 1)]  # (pair, tci) per unit idx
        ustate = [
            {"pair": p, "tci": t, "pvs": [None, None], "pp": None}
            for (p, t) in units
        ]

        def get_pp(ui):
            if ustate[ui]["pp"] is None:
                ustate[ui]["pp"] = pppool.tile(
                    [P, 2, S_TILES, TQ], BF16, name="pp", tag="pp")
            return ustate[ui]["pp"]

        ebtiles = {}

        def emit_eb(ui, g, j):
            pair, tci = units[ui]
            ebt = ebpool.tile([P, 2 * TQ], BF16, name="eb", tag="eb")
            nc.sync.dma_start(ebt[:], eb[2 * pair + j, tci, g])
            ebtiles[(ui, g, j)] = ebt

        def emit_pv(ui, j, m_lo, m_hi):
            st = ustate[ui]
            pp = get_pp(ui)
            if st["pvs"][j] is None:
                st["pvs"][j] = psV.tile(
                    [P, TQ], F32, name=f"pv{j}", tag=f"pv{j}")
            h = 2 * st["pair"] + j
            for m in range(m_lo, m_hi + 1):
                nc.tensor.matmul(
                    st["pvs"][j][:],
                    lhsT=v_sb[:, m, h, :],
                    rhs=pp[:, j, m, :],
                    start=(m == 0),
                    stop=(m == S_TILES - 1),
                )

        def norm_j(ui, j):
            """1/den on DVE, then one DVE mul -> attn."""
            st = ustate[ui]
            pair, tci = st["pair"], st["tci"]
            t_sl = slice(tci * TQ, (tci + 1) * TQ)
            p0 = j * HEAD_DIM
            rc = rcpool.tile([P, TQ], F32, name="rc", tag="rc")
            nc.vector.reciprocal(
                rc[HEAD_DIM:P, :], st["pvs"][j][HEAD_DIM:P, :])
            nc.vector.tensor_mul(
                attn_sb[p0:p0 + HEAD_DIM, pair, t_sl],
                st["pvs"][j][0:HEAD_DIM, :],
                rc[HEAD_DIM:P, :],
            )
            st["pvs"][j] = None

        def attn_qk_group(ui, g):
            """QK for group g of unit ui + exp + eb-multiply."""
            st = ustate[ui]
            pair, tci = st["pair"], st["tci"]
            pp = get_pp(ui)
            ms = 2 * g
            t_sl = slice(tci * TQ, (tci + 1) * TQ)
            plss = []
            for j in range(2):
                tag = (2 * g + j) % 2
                plss.append(psL.tile([P, 2, TQ], F32, name=f"lg{tag}",
                                     tag=f"lg{tag}"))
            # j-adjacent issue: the two K=64 matmuls run in disjoint PE
            # row halves
            for mi in range(2):
                for j in range(2):
                    p0 = j * HEAD_DIM
                    nc.tensor.matmul(
                        plss[j][:, mi, :],
                        lhsT=k_sb[p0:p0 + HEAD_DIM, pair,
                                  (ms + mi) * P:(ms + mi + 1) * P],
                        rhs=q_sb[p0:p0 + HEAD_DIM, pair, t_sl],
                        start=True,
                        stop=True,
                    )
            for j in range(2):
                tag = (2 * g + j) % 2
                pg = pgpool.tile([P, 2, TQ], BF16, name=f"pg{tag}",
                                 tag=f"pg{tag}")
                nc.scalar.activation(pg[:], plss[j][:], Exp)
                ebt = ebtiles.pop((ui, g, j))
                nc.vector.tensor_mul(
                    pp[:, j, ms:ms + 2, :],
                    pg[:],
                    ebt.rearrange("p (m t) -> p m t", t=TQ),
                )

        # ---- out projection (partial; host sums head groups) ----
        def outproj_pair(tci, mo0, copy_engine):
            for mo in (mo0, mo0 + 1):
                ps = mm_slots()
                for kt in range(DT):
                    nc.tensor.matmul(
                        ps[:],
                        lhsT=wo_sb[:, kt, mo * P:(mo + 1) * P],
                        rhs=attn_sb[:, kt, tci * TQ:(tci + 1) * TQ],
                        start=(kt == 0),
                        stop=(kt == DT - 1),
                    )
                ot = outp.tile([P, TQ], BF16, name="ot")
                if copy_engine == "vector":
                    nc.vector.tensor_copy(ot[:], ps[:])
                else:
                    nc.scalar.activation(ot[:], ps[:], Copy)
                nc.sync.dma_start(outT[mo, tci], ot[:])

        # ================= emission schedule =================
        with nc.named_scope("pre"):
            # DMA priority order: everything the preamble + window 0 needs
            nc.sync.dma_start(wq_sb[:], wq)
            xqt0 = x_chunk(xqpool, xq, 0, "xqt")
            nc.sync.dma_start(wv_sb[:], wv)
            xvt = [x_chunk(xvpool, xv, g, "xvt") for g in range(2)]
            nc.sync.dma_start(wk_sb[:], wk)
            xkt = [x_chunk(xkpool, xk, c, "xkt") for c in range(2)]

            proj_chunk(xqt0, wq_sb, q_sb, 0, pre_slots)
            proj_v_group(xvt[0], 0, pre_slots)
            xvt.append(x_chunk(xvpool, xv, 2, "xvt"))
            proj_v_group(xvt[1], 1, pre_slots)
            proj_chunk(xkt[0], wk_sb, k_sb, 0, pre_slots)
            xkt.append(x_chunk(xkpool, xk, 2, "xkt"))
            proj_chunk(xkt[1], wk_sb, k_sb, 1, pre_slots)
            nc.sync.dma_start(wo_sb[:], wo)
            # exp'd-bias stream: 4 groups of lead, advanced 2 tiles per
            # attention group below (in-flight stays at 8 <= ebpool bufs,
            # so no DMA ever queues behind an sem-gated transfer)
            for g in range(4):
                emit_eb(0, g, 0)
                emit_eb(0, g, 1)

        # aux work emitted at the START of (window, group)
        xqt1 = []

        def aux(w, g):
            if w == 0:
                if g == 0:
                    xkt.append(x_chunk(xkpool, xk, 3, "xkt"))
                elif g == 1:
                    xvt.append(x_chunk(xvpool, xv, 3, "xvt"))
                elif g == 2:
                    proj_chunk(xkt[2], wk_sb, k_sb, 2, mm_slots)
                    proj_v_group(xvt[2], 2, mm_slots)
                elif g == 4:
                    proj_chunk(xkt[3], wk_sb, k_sb, 3, mm_slots)
                elif g == 5:
                    proj_v_group(xvt[3], 3, mm_slots)
                elif g == 6:
                    xqt1.append(x_chunk(xqpool, xq, 1, "xqt"))
            elif w == 1 and g == 2:
                proj_chunk(xqt1[0], wq_sb, q_sb, 1, mm_slots)
            elif w == 2 and g >= 4:
                outproj_pair(0, 2 * (g - 4), "vector")

        for w in range(4):
            with nc.named_scope(f"u{w}"):
                for g in range(NG):
                    # advance the eb stream cursor (4 groups ahead)
                    gg = 8 * w + g + 4
                    if gg < 32:
                        emit_eb(gg // 8, gg % 8, 0)
                        emit_eb(gg // 8, gg % 8, 1)
                    for (ui, j) in NORM_TABLE.get(w, {}).get(g, []):
                        norm_j(ui, j)
                    for (ui, j, mlo, mhi) in PV_TABLE.get(w, {}).get(g, []):
                        emit_pv(ui, j, mlo, mhi)
                    aux(w, g)
                    attn_qk_group(w, g)

        with nc.named_scope("tail"):
            emit_pv(3, 0, 14, 15)
            emit_pv(3, 1, 14, 15)
            norm_j(3, 0)
            norm_j(3, 1)
            for mo0 in range(0, H_DIM // P, 2):
                outproj_pair(1, mo0, "scalar")


def _build_program():
    key = ("prog", "bf16_v6")
    if key in _prog_cache:
        return _prog_cache[key]
    nc = bacc.Bacc("TRN2", target_bir_lowering=False, debug=False,
                   num_devices=N_CORES)
    ins = [
        nc.dram_tensor("xq", [KT, NQ, P, TQ], BF16, kind="ExternalInput").ap(),
        nc.dram_tensor("xk", [KT, NKC, P, TQ], BF16, kind="ExternalInput").ap(),
        nc.dram_tensor("xv", [KT, VG, P, TQ], BF16, kind="ExternalInput").ap(),
        nc.dram_tensor("eb", [HPC, NQ, NG, P, 2 * TQ], BF16,
                       kind="ExternalInput").ap(),
        nc.dram_tensor("wq", [P, KT, DPC], BF16, kind="ExternalInput").ap(),
        nc.dram_tensor("wk", [P, KT, DPC], BF16, kind="ExternalInput").ap(),
        nc.dram_tensor("wv", [P, KT, DPC], BF16, kind="ExternalInput").ap(),
        nc.dram_tensor("wo", [P, DT, H_DIM], BF16, kind="ExternalInput").ap(),
    ]
    outs = [nc.dram_tensor("outT", [H_DIM // P, NQ, P, TQ], BF16,
                           kind="ExternalOutput").ap()]
    with tile.TileContext(nc) as tc:
        _emit(tc, outs, ins)
    nc.compile()
    _prog_cache[key] = nc
    return nc


def _tile_x(xT):
    """[E, L] -> [KT, L//TQ, P, TQ] contiguous tiles."""
    E, L = xT.shape
    return np.ascontiguousarray(
        xT.reshape(KT, P, L // TQ, TQ).transpose(0, 2, 1, 3)).astype(NPBF16)


def _host_prep(query, key, value, attn_bias, attention_mask,
               Wq, bq, Wk, bk, Wv, bv, Wo, bo):
    """Build the 8 per-core input maps (all bf16, pre-tiled)."""
    f = np.float32
    query = np.asarray(query, f)
    key = np.asarray(key, f)
    value = np.asarray(value, f)
    attn_bias = np.asarray(attn_bias, f)
    mask = np.asarray(attention_mask)
    Wq = np.asarray(Wq, f); bq = np.asarray(bq, f)
    Wk = np.asarray(Wk, f)
    Wv = np.asarray(Wv, f)
    Wo = np.asarray(Wo, f)

    scale = f(1.0 / np.sqrt(HEAD_DIM))
    # c[b, s, h] = scale * (bq_h . k_h(s)) with k = key @ Wk^T (no bk —
    # bk cancels in softmax). U[e, h] = sum_{d in head h} Wk[d, e] bq[d].
    U = (Wk * (bq * scale)[:, None]).reshape(N_HEADS, HEAD_DIM, H_DIM)
    U = U.sum(axis=1)  # [H, E]
    c = np.einsum("bse,he->bsh", key, U)  # [B, S, H]

    # exp'd masked bias: eb[b,h,s,t] = exp(bias[b,h,t,s] + c[b,s,h]); 0 masked
    ebias = np.exp(attn_bias.transpose(0, 1, 3, 2)
                   + c.transpose(0, 2, 1)[:, :, :, None])
    maskT = mask.transpose(0, 2, 1)[:, None, :, :]  # [B, 1, S, T]
    ebias = np.where(maskT, f(0.0), ebias)
    # tile: [B, H, S, T] -> [B, H, NQ, NG(g), P, (mm, t)]
    # s = g*256 + mm*128 + p ; t = tci*TQ + tt
    ebias = ebias.reshape(B, N_HEADS, NG, 2, P, NQ, TQ)
    # axes: [b, h, g, mm, p, tci, tt] -> [b, h, tci, g, p, mm, tt]
    ebias = np.ascontiguousarray(
        ebias.transpose(0, 1, 5, 2, 4, 3, 6)).reshape(
        B, N_HEADS, NQ, NG, P, 2 * TQ).astype(NPBF16)

    xqT = [_tile_x(query[b].T) for b in range(B)]
    xkT = [_tile_x(key[b].T) for b in range(B)]
    xvT = [_tile_x(value[b].T) for b in range(B)]

    def tile_w(wT):  # [E=1024, D=256] -> [128, 8, 256]
        return np.ascontiguousarray(
            wT.reshape(KT, P, DPC).transpose(1, 0, 2)).astype(NPBF16)

    in_maps = []
    for cc in range(N_CORES):
        b, g = divmod(cc, N_CORES // B)
        hs = g * HPC
        he = hs + HPC
        ds_, de = hs * HEAD_DIM, he * HEAD_DIM
        in_maps.append({
            "xq": xqT[b],
            "xk": xkT[b],
            "xv": xvT[b],
            "eb": np.ascontiguousarray(ebias[b, hs:he]),
            "wq": tile_w((Wq[ds_:de] * scale).T),
            "wk": tile_w(Wk[ds_:de].T),
            "wv": tile_w(Wv[ds_:de].T),
            "wo": np.ascontiguousarray(
                Wo[:, ds_:de].T.reshape(DT, P, H_DIM).transpose(1, 0, 2)
            ).astype(NPBF16),
        })
    return in_maps


def _assemble(results, Wo, bv, bo):
    Wo = np.asarray(Wo, np.float64)
    bv = np.asarray(bv, np.float64)
    bo = np.asarray(bo, np.float64)
    bconst = Wo @ bv + bo  # [H_DIM]
    G = N_CORES // B
    out = np.empty((B, TGT, H_DIM), np.float32)
    for b in range(B):
        acc = np.zeros((H_DIM, TGT), np.float64)
        for g in range(G):
            blk = np.asarray(results[b * G + g]["outT"], np.float32)
            acc += blk.transpose(0, 2, 1, 3).reshape(H_DIM, TGT)
        out[b] = (acc.T + bconst[None, :]).astype(np.float32)
    return out


def kernel(**inputs):
    in_maps = _host_prep(**inputs)
    nc = _build_program()
    res = run_bass_kernel_spmd(nc, in_maps, core_ids=list(range(N_CORES)))
    return _assemble(res.results, inputs["Wo"], inputs["bv"], inputs["bo"])


# revision 13
# speedup vs baseline: 1.3355x; 1.3355x over previous
"""Trainium2 Bass kernel for nn_CrossAttention (B=2, TGT=1024, SRC=2048,
H=1024, 16 heads x 64).

Sharding: 8 cores = 2 (batch) x 4 (head groups of 4 heads). Each core
computes q/k/v projections for its 4 heads (column-sliced weights), the
attention for those heads, and a partial out-projection (row-sliced Wo).
The host sums the 4 partial out-projections per batch and adds bo.

v6 (scheduling rewrite of the v5 baseline; same all-bf16 math):
  * The kernel is PE-bound (~96us of matmul rows at 2.4GHz) with the DMA
    stream (~31.5MB at ~330GB/s) just underneath.  The v5 baseline lost
    ~60us to HAM cold-clock oscillation (PE idle gaps re-throttle the PE
    clock to 1.2GHz), a 15us startup, and a 12us serial tail.
  * PV software pipeline is restructured so each unit's PV drains as
    early as possible: unit u's j0 half self-drains inside its own
    window (groups 3-7 + 2 leftovers), j1 drains in the first 3 groups
    of the next window.  Softmax norms then land 1-3 groups into the
    next window instead of at its end, which unblocks the out-projection
    (interleaved into window 2) and shrinks the tail to 4 PV matmuls +
    norms + outproj_t1.
  * Projections are interleaved at chunk granularity into the attention
    windows (psA copy latency hides under psL/psV matmuls), x inputs are
    whole-chunk SBUF tiles whose DMAs issue far ahead, and the exp'd
    bias stream prefetches a full unit ahead through a 10-deep pool.
  * Preamble projections cycle their PSUM tiles through the (still
    unused) psV slots as well as psA so back-to-back projection chunks
    never stall on the PSUM->SBUF copy.
  * All projection biases fold away exactly as in v5 (bk cancels, bq
    folds into the host-exponentiated bias, Wo@bv+bo added on host).
"""

import numpy as np
from contextlib import ExitStack

import ml_dtypes

import concourse.bass as bass
import concourse.tile as tile
from concourse import bacc, mybir
from concourse.bass_utils import run_bass_kernel_spmd

P = 128
H_DIM = 1024
N_HEADS = 16
HEAD_DIM = 64
B = 2
TGT = 1024
SRC = 2048
N_CORES = 8
HPC = 4  # heads per core
DPC = HPC * HEAD_DIM  # 256 projected dims per core
F32 = mybir.dt.float32
BF16 = mybir.dt.bfloat16
NPBF16 = ml_dtypes.bfloat16

TQ = 512  # t-chunk for attention units
S_TILES = SRC // P  # 16
KT = H_DIM // P  # 8 contraction tiles for projections
DT = DPC // P  # 2 d-tiles per core
NQ = TGT // TQ  # 2 t-chunks
NKC = SRC // TQ  # 4 n-chunks for k proj
VG = 4  # m-tile groups for v proj (4 s-tiles each)
NG = S_TILES // 2  # 8 attention groups per unit

# window -> group -> list of (owner_unit, j, m_lo, m_hi) PV drains
PV_TABLE = {
    1: {0: [(0, 0, 0, 7)], 1: [(0, 0, 8, 15)], 2: [(0, 1, 0, 7)],
        3: [(0, 1, 8, 15)], 4: [(1, 0, 0, 3)], 5: [(1, 0, 4, 7)],
        6: [(1, 0, 8, 11)], 7: [(1, 0, 12, 13)]},
    2: {0: [(1, 0, 14, 15), (1, 1, 0, 5)], 1: [(1, 1, 6, 11)],
        2: [(1, 1, 12, 15)], 3: [(2, 0, 0, 3)], 4: [(2, 0, 4, 7)],
        5: [(2, 0, 8, 9)], 6: [(2, 0, 10, 11)], 7: [(2, 0, 12, 13)]},
    3: {0: [(2, 0, 14, 15), (2, 1, 0, 5)], 1: [(2, 1, 6, 11)],
        2: [(2, 1, 12, 15)], 3: [(3, 0, 0, 3), (3, 1, 0, 1)],
        4: [(3, 0, 4, 7), (3, 1, 2, 5)], 5: [(3, 0, 8, 9), (3, 1, 6, 9)],
        6: [(3, 0, 10, 11), (3, 1, 10, 11)],
        7: [(3, 0, 12, 13), (3, 1, 12, 13)]},
}
# window -> group -> (owner_unit, j) norms (emitted before that group's PVs)
NORM_TABLE = {
    1: {2: [(0, 0)], 4: [(0, 1)]},
    2: {1: [(1, 0)], 3: [(1, 1)]},
    3: {1: [(2, 0)], 3: [(2, 1)]},
}

_prog_cache: dict = {}


def _emit(tc: tile.TileContext, outs, ins):
    nc = tc.nc
    xq, xk, xv, eb, wq, wk, wv, wo = ins
    (outT,) = outs
    Exp = mybir.ActivationFunctionType.Exp
    Copy = mybir.ActivationFunctionType.Copy

    with ExitStack() as ctx:
        const = ctx.enter_context(tc.tile_pool(name="const", bufs=1))
        xqpool = ctx.enter_context(tc.tile_pool(name="xqin", bufs=2))
        xkpool = ctx.enter_context(tc.tile_pool(name="xkin", bufs=2))
        xvpool = ctx.enter_context(tc.tile_pool(name="xvin", bufs=3))
        ebpool = ctx.enter_context(tc.tile_pool(name="ebin", bufs=10))
        pgpool = ctx.enter_context(tc.tile_pool(name="pg", bufs=2))
        pppool = ctx.enter_context(tc.tile_pool(name="pp", bufs=2))
        rcpool = ctx.enter_context(tc.tile_pool(name="rcp", bufs=2))
        outp = ctx.enter_context(tc.tile_pool(name="outsb", bufs=3))
        psA = ctx.enter_context(tc.tile_pool(name="psA", bufs=2, space="PSUM"))
        psL = ctx.enter_context(tc.tile_pool(name="psL", bufs=1, space="PSUM"))
        psV = ctx.enter_context(tc.tile_pool(name="psV", bufs=1, space="PSUM"))

        # ---- persistent SBUF tensors ----
        wq_sb = const.tile([P, KT, DPC], BF16)  # [e_part, e_tile, d]
        wk_sb = const.tile([P, KT, DPC], BF16)
        wv_sb = const.tile([P, KT, DPC], BF16)
        wo_sb = const.tile([P, DT, H_DIM], BF16)  # [hd_part, hd_tile, e_out]
        q_sb = const.tile([P, DT, TGT], BF16)  # qT
        k_sb = const.tile([P, DT, SRC], BF16)  # kT
        # v plus 64 ones-columns, per (s_tile, head): [.., 0:64]=v, [.., 64:128]=1
        v_sb = const.tile([P, S_TILES, HPC, P], BF16)
        attn_sb = const.tile([P, DT, TGT], BF16)  # attnT, normalized

        nc.gpsimd.memset(v_sb[:, :, :, HEAD_DIM:P], 1.0)

        # ---- x chunk tiles (one DMA batch per chunk, deep prefetch) ----
        def x_chunk(pool, x_dram, n, nm):
            xt = pool.tile([P, KT, TQ], BF16, name=nm)
            for k in range(KT):
                nc.sync.dma_start(xt[:, k, :], x_dram[k, n])
            return xt

        # ---- q/k projections: psum[d_tile] += wT_tile.T @ xT_tile ----
        def proj_chunk(xt, w_sb, dst_sb, n, slots):
            pss = [slots() for _ in range(DT)]
            for k in range(KT):
                for m in range(DT):
                    nc.tensor.matmul(
                        pss[m][:],
                        lhsT=w_sb[:, k, m * P:(m + 1) * P],
                        rhs=xt[:, k, :],
                        start=(k == 0),
                        stop=(k == KT - 1),
                    )
            for m in range(DT):
                nc.vector.tensor_copy(
                    dst_sb[:, m, n * TQ:(n + 1) * TQ], pss[m][:])

        # ---- v projection group: 4 s-tiles, all heads ----
        def proj_v_group(xvt, mg, slots):
            for ml in range(VG):
                m = mg * VG + ml
                ps = slots()[:, :DPC]
                for k in range(KT):
                    nc.tensor.matmul(
                        ps,
                        lhsT=xvt[:, k, ml * P:(ml + 1) * P],
                        rhs=wv_sb[:, k, :],
                        start=(k == 0),
                        stop=(k == KT - 1),
                    )
                nc.vector.tensor_copy(
                    v_sb[:, m, :, 0:HEAD_DIM],
                    ps.rearrange("p (h d) -> p h d", d=HEAD_DIM),
                )

        # psum slot cyclers.  Preamble rotates through psA + the not-yet-
        # used psV slots so chunk copies never stall; in-window projs and
        # the out-projection use psA only.
        class Slots:
            def __init__(self, seq):
                self.seq = seq
                self.i = 0

            def __call__(self):
                pool, tag = self.seq[self.i % len(self.seq)]
                self.i += 1
                t = pool.tile([P, TQ], F32, name=f"ps_{tag}", tag=tag)
                return t

        pre_slots = Slots([(psA, "mm"), (psV, "pv0"), (psV, "pv1"), (psA, "mm")])
        mm_slots = Slots([(psA, "mm")])

        # ---- attention machinery ----
        units = [(0, 0), (1, 0), (0, 1), (1, 1)]  # (pair, tci) per unit idx
        ustate = [
            {"pair": p, "tci": t, "pvs": [None, None], "pp": None}
            for (p, t) in units
        ]

        def get_pp(ui):
            if ustate[ui]["pp"] is None:
                ustate[ui]["pp"] = pppool.tile(
                    [P, 2, S_TILES, TQ], BF16, name="pp", tag="pp")
            return ustate[ui]["pp"]

        ebtiles = {}

        def emit_eb(ui, g, j):
            pair, tci = units[ui]
            ebt = ebpool.tile([P, 2 * TQ], BF16, name="eb", tag="eb")
            nc.sync.dma_start(ebt[:], eb[2 * pair + j, tci, g])
            ebtiles[(ui, g, j)] = ebt

        def emit_pv(ui, j, m_lo, m_hi):
            st = ustate[ui]
            pp = get_pp(ui)
            if st["pvs"][j] is None:
                st["pvs"][j] = psV.tile(
                    [P, TQ], F32, name=f"pv{j}", tag=f"pv{j}")
            h = 2 * st["pair"] + j
            for m in range(m_lo, m_hi + 1):
                nc.tensor.matmul(
                    st["pvs"][j][:],
                    lhsT=v_sb[:, m, h, :],
                    rhs=pp[:, j, m, :],
                    start=(m == 0),
                    stop=(m == S_TILES - 1),
                )

        def norm_j(ui, j):
            """1/den on DVE (fast Newton-Raphson approx), one DVE mul -> attn.
            The exact `reciprocal` is 8 cyc/elem (~4us per norm) and stalls
            the in-order DVE queue behind it."""
            st = ustate[ui]
            pair, tci = st["pair"], st["tci"]
            t_sl = slice(tci * TQ, (tci + 1) * TQ)
            p0 = j * HEAD_DIM
            rc = rcpool.tile([P, TQ], F32, name="rc", tag="rc")
            # the custom-DVE approx mishandles partition-base offsets, so
            # run it over all 128 partitions; rows 0-63 (1/numerator) are
            # junk and never read
            nc.vector.reciprocal_approx_fast(rc[:], st["pvs"][j][:])
            nc.vector.tensor_mul(
                attn_sb[p0:p0 + HEAD_DIM, pair, t_sl],
                st["pvs"][j][0:HEAD_DIM, :],
                rc[HEAD_DIM:P, :],
            )
            st["pvs"][j] = None

        def attn_qk_group(ui, g):
            """QK for group g of unit ui + exp + eb-multiply."""
            st = ustate[ui]
            pair, tci = st["pair"], st["tci"]
            pp = get_pp(ui)
            ms = 2 * g
            t_sl = slice(tci * TQ, (tci + 1) * TQ)
            plss = []
            for j in range(2):
                tag = (2 * g + j) % 2
                plss.append(psL.tile([P, 2, TQ], F32, name=f"lg{tag}",
                                     tag=f"lg{tag}"))
            # j-adjacent issue: the two K=64 matmuls run in disjoint PE
            # row halves
            for mi in range(2):
                for j in range(2):
                    p0 = j * HEAD_DIM
                    nc.tensor.matmul(
                        plss[j][:, mi, :],
                        lhsT=k_sb[p0:p0 + HEAD_DIM, pair,
                                  (ms + mi) * P:(ms + mi + 1) * P],
                        rhs=q_sb[p0:p0 + HEAD_DIM, pair, t_sl],
                        start=True,
                        stop=True,
                    )
            for j in range(2):
                tag = (2 * g + j) % 2
                pg = pgpool.tile([P, 2, TQ], BF16, name=f"pg{tag}",
                                 tag=f"pg{tag}")
                nc.scalar.activation(pg[:], plss[j][:], Exp)
                ebt = ebtiles.pop((ui, g, j))
                nc.vector.tensor_mul(
                    pp[:, j, ms:ms + 2, :],
                    pg[:],
                    ebt.rearrange("p (m t) -> p m t", t=TQ),
                )

        # ---- out projection (partial; host sums head groups) ----
        def outproj_pair(tci, mo0, copy_engine):
            for mo in (mo0, mo0 + 1):
                ps = mm_slots()
                for kt in range(DT):
                    nc.tensor.matmul(
                        ps[:],
                        lhsT=wo_sb[:, kt, mo * P:(mo + 1) * P],
                        rhs=attn_sb[:, kt, tci * TQ:(tci + 1) * TQ],
                        start=(kt == 0),
                        stop=(kt == DT - 1),
                    )
                ot = outp.tile([P, TQ], BF16, name="ot")
                if copy_engine == "vector":
                    nc.vector.tensor_copy(ot[:], ps[:])
                else:
                    nc.scalar.activation(ot[:], ps[:], Copy)
                nc.sync.dma_start(outT[mo, tci], ot[:])

        # ================= emission schedule =================
        # The attention stream (QK -> exp -> eb-mul) is the critical chain:
        # the ACT exp ops (1.3us each, 2 per group) pace the whole kernel.
        # So: minimal preamble (q-t0 + k chunk 0 only -> first QK at ~13us),
        # then every other projection / out-projection chunk rides in the
        # PE slack after its window-group's QK.
        with nc.named_scope("pre"):
            nc.sync.dma_start(wq_sb[:], wq)
            xqt0 = x_chunk(xqpool, xq, 0, "xqt")
            nc.sync.dma_start(wk_sb[:], wk)
            xkt = [x_chunk(xkpool, xk, 0, "xkt"), x_chunk(xkpool, xk, 1, "xkt")]
            nc.sync.dma_start(wv_sb[:], wv)
            nc.sync.dma_start(wo_sb[:], wo)
            proj_chunk(xqt0, wq_sb, q_sb, 0, pre_slots)
            proj_chunk(xkt[0], wk_sb, k_sb, 0, pre_slots)
            for g in range(4):
                emit_eb(0, g, 0)
                emit_eb(0, g, 1)

        # aux work emitted AFTER (window, group)'s QK so a straggling input
        # DMA stalls the PE only where the exp stream covers it
        xvt, xqt1 = [], []

        def aux(w, g):
            if w == 0:
                if g == 0:
                    xvt.append(x_chunk(xvpool, xv, 0, "xvt"))
                    xvt.append(x_chunk(xvpool, xv, 1, "xvt"))
                elif g == 1:
                    proj_chunk(xkt[1], wk_sb, k_sb, 1, mm_slots)
                    xkt.append(x_chunk(xkpool, xk, 2, "xkt"))
                elif g == 2:
                    proj_v_group(xvt[0], 0, mm_slots)
                    xvt.append(x_chunk(xvpool, xv, 2, "xvt"))
                elif g == 3:
                    proj_chunk(xkt[2], wk_sb, k_sb, 2, mm_slots)
                    xkt.append(x_chunk(xkpool, xk, 3, "xkt"))
                elif g == 4:
                    proj_v_group(xvt[1], 1, mm_slots)
                    xvt.append(x_chunk(xvpool, xv, 3, "xvt"))
                elif g == 5:
                    proj_chunk(xkt[3], wk_sb, k_sb, 3, mm_slots)
                elif g == 6:
                    proj_v_group(xvt[2], 2, mm_slots)
                    xqt1.append(x_chunk(xqpool, xq, 1, "xqt"))
            elif w == 1:
                if g == 0:
                    proj_v_group(xvt[3], 3, mm_slots)
                elif g == 2:
                    proj_chunk(xqt1[0], wq_sb, q_sb, 1, mm_slots)
            elif w == 2 and g >= 4:
                outproj_pair(0, 2 * (g - 4), "vector")

        for w in range(4):
            with nc.named_scope(f"u{w}"):
                for g in range(NG):
                    # advance the eb stream cursor (4 groups ahead)
                    gg = 8 * w + g + 4
                    if gg < 32:
                        emit_eb(gg // 8, gg % 8, 0)
                        emit_eb(gg // 8, gg % 8, 1)
                    for (ui, j) in NORM_TABLE.get(w, {}).get(g, []):
                        norm_j(ui, j)
                    for (ui, j, mlo, mhi) in PV_TABLE.get(w, {}).get(g, []):
                        emit_pv(ui, j, mlo, mhi)
                    attn_qk_group(w, g)
                    aux(w, g)

        with nc.named_scope("tail"):
            emit_pv(3, 0, 14, 15)
            emit_pv(3, 1, 14, 15)
            norm_j(3, 0)
            norm_j(3, 1)
            for mo0 in range(0, H_DIM // P, 2):
                outproj_pair(1, mo0, "scalar")


def _build_program():
    key = ("prog", "bf16_v6")
    if key in _prog_cache:
        return _prog_cache[key]
    nc = bacc.Bacc("TRN2", target_bir_lowering=False, debug=False,
                   num_devices=N_CORES)
    ins = [
        nc.dram_tensor("xq", [KT, NQ, P, TQ], BF16, kind="ExternalInput").ap(),
        nc.dram_tensor("xk", [KT, NKC, P, TQ], BF16, kind="ExternalInput").ap(),
        nc.dram_tensor("xv", [KT, VG, P, TQ], BF16, kind="ExternalInput").ap(),
        nc.dram_tensor("eb", [HPC, NQ, NG, P, 2 * TQ], BF16,
                       kind="ExternalInput").ap(),
        nc.dram_tensor("wq", [P, KT, DPC], BF16, kind="ExternalInput").ap(),
        nc.dram_tensor("wk", [P, KT, DPC], BF16, kind="ExternalInput").ap(),
        nc.dram_tensor("wv", [P, KT, DPC], BF16, kind="ExternalInput").ap(),
        nc.dram_tensor("wo", [P, DT, H_DIM], BF16, kind="ExternalInput").ap(),
    ]
    outs = [nc.dram_tensor("outT", [H_DIM // P, NQ, P, TQ], BF16,
                           kind="ExternalOutput").ap()]
    with tile.TileContext(nc) as tc:
        _emit(tc, outs, ins)
    nc.compile()
    _prog_cache[key] = nc
    return nc


def _tile_x(xT):
    """[E, L] -> [KT, L//TQ, P, TQ] contiguous tiles."""
    E, L = xT.shape
    return np.ascontiguousarray(
        xT.reshape(KT, P, L // TQ, TQ).transpose(0, 2, 1, 3)).astype(NPBF16)


def _host_prep(query, key, value, attn_bias, attention_mask,
               Wq, bq, Wk, bk, Wv, bv, Wo, bo):
    """Build the 8 per-core input maps (all bf16, pre-tiled)."""
    f = np.float32
    query = np.asarray(query, f)
    key = np.asarray(key, f)
    value = np.asarray(value, f)
    attn_bias = np.asarray(attn_bias, f)
    mask = np.asarray(attention_mask)
    Wq = np.asarray(Wq, f); bq = np.asarray(bq, f)
    Wk = np.asarray(Wk, f)
    Wv = np.asarray(Wv, f)
    Wo = np.asarray(Wo, f)

    scale = f(1.0 / np.sqrt(HEAD_DIM))
    # c[b, s, h] = scale * (bq_h . k_h(s)) with k = key @ Wk^T (no bk —
    # bk cancels in softmax). U[e, h] = sum_{d in head h} Wk[d, e] bq[d].
    U = (Wk * (bq * scale)[:, None]).reshape(N_HEADS, HEAD_DIM, H_DIM)
    U = U.sum(axis=1)  # [H, E]
    c = np.einsum("bse,he->bsh", key, U)  # [B, S, H]

    # exp'd masked bias: eb[b,h,s,t] = exp(bias[b,h,t,s] + c[b,s,h]); 0 masked
    ebias = np.exp(attn_bias.transpose(0, 1, 3, 2)
                   + c.transpose(0, 2, 1)[:, :, :, None])
    maskT = mask.transpose(0, 2, 1)[:, None, :, :]  # [B, 1, S, T]
    ebias = np.where(maskT, f(0.0), ebias)
    # tile: [B, H, S, T] -> [B, H, NQ, NG(g), P, (mm, t)]
    # s = g*256 + mm*128 + p ; t = tci*TQ + tt
    ebias = ebias.reshape(B, N_HEADS, NG, 2, P, NQ, TQ)
    # axes: [b, h, g, mm, p, tci, tt] -> [b, h, tci, g, p, mm, tt]
    ebias = np.ascontiguousarray(
        ebias.transpose(0, 1, 5, 2, 4, 3, 6)).reshape(
        B, N_HEADS, NQ, NG, P, 2 * TQ).astype(NPBF16)

    xqT = [_tile_x(query[b].T) for b in range(B)]
    xkT = [_tile_x(key[b].T) for b in range(B)]
    xvT = [_tile_x(value[b].T) for b in range(B)]

    def tile_w(wT):  # [E=1024, D=256] -> [128, 8, 256]
        return np.ascontiguousarray(
            wT.reshape(KT, P, DPC).transpose(1, 0, 2)).astype(NPBF16)

    in_maps = []
    for cc in range(N_CORES):
        b, g = divmod(cc, N_CORES // B)
        hs = g * HPC
        he = hs + HPC
        ds_, de = hs * HEAD_DIM, he * HEAD_DIM
        in_maps.append({
            "xq": xqT[b],
            "xk": xkT[b],
            "xv": xvT[b],
            "eb": np.ascontiguousarray(ebias[b, hs:he]),
            "wq": tile_w((Wq[ds_:de] * scale).T),
            "wk": tile_w(Wk[ds_:de].T),
            "wv": tile_w(Wv[ds_:de].T),
            "wo": np.ascontiguousarray(
                Wo[:, ds_:de].T.reshape(DT, P, H_DIM).transpose(1, 0, 2)
            ).astype(NPBF16),
        })
    return in_maps


def _assemble(results, Wo, bv, bo):
    Wo = np.asarray(Wo, np.float64)
    bv = np.asarray(bv, np.float64)
    bo = np.asarray(bo, np.float64)
    bconst = Wo @ bv + bo  # [H_DIM]
    G = N_CORES // B
    out = np.empty((B, TGT, H_DIM), np.float32)
    for b in range(B):
        acc = np.zeros((H_DIM, TGT), np.float64)
        for g in range(G):
            blk = np.asarray(results[b * G + g]["outT"], np.float32)
            acc += blk.transpose(0, 2, 1, 3).reshape(H_DIM, TGT)
        out[b] = (acc.T + bconst[None, :]).astype(np.float32)
    return out


def kernel(**inputs):
    in_maps = _host_prep(**inputs)
    nc = _build_program()
    res = run_bass_kernel_spmd(nc, in_maps, core_ids=list(range(N_CORES)))
    return _assemble(res.results, inputs["Wo"], inputs["bv"], inputs["bo"])


# revision 15
# speedup vs baseline: 1.3779x; 1.0318x over previous
"""Trainium2 Bass kernel for nn_CrossAttention (B=2, TGT=1024, SRC=2048,
H=1024, 16 heads x 64).

Sharding: 8 cores = 2 (batch) x 4 (head groups of 4 heads). Each core
computes q/k/v projections for its 4 heads (column-sliced weights), the
attention for those heads, and a partial out-projection (row-sliced Wo).
The host sums the 4 partial out-projections per batch and adds bo.

v6 (scheduling rewrite of the v5 baseline; same all-bf16 math):
  * The kernel is PE-bound (~96us of matmul rows at 2.4GHz) with the DMA
    stream (~31.5MB at ~330GB/s) just underneath.  The v5 baseline lost
    ~60us to HAM cold-clock oscillation (PE idle gaps re-throttle the PE
    clock to 1.2GHz), a 15us startup, and a 12us serial tail.
  * PV software pipeline is restructured so each unit's PV drains as
    early as possible: unit u's j0 half self-drains inside its own
    window (groups 3-7 + 2 leftovers), j1 drains in the first 3 groups
    of the next window.  Softmax norms then land 1-3 groups into the
    next window instead of at its end, which unblocks the out-projection
    (interleaved into window 2) and shrinks the tail to 4 PV matmuls +
    norms + outproj_t1.
  * Projections are interleaved at chunk granularity into the attention
    windows (psA copy latency hides under psL/psV matmuls), x inputs are
    whole-chunk SBUF tiles whose DMAs issue far ahead, and the exp'd
    bias stream prefetches a full unit ahead through a 10-deep pool.
  * Preamble projections cycle their PSUM tiles through the (still
    unused) psV slots as well as psA so back-to-back projection chunks
    never stall on the PSUM->SBUF copy.
  * All projection biases fold away exactly as in v5 (bk cancels, bq
    folds into the host-exponentiated bias, Wo@bv+bo added on host).
"""

import numpy as np
from contextlib import ExitStack

import ml_dtypes

import concourse.bass as bass
import concourse.tile as tile
from concourse import bacc, mybir
from concourse.bass_utils import run_bass_kernel_spmd

P = 128
H_DIM = 1024
N_HEADS = 16
HEAD_DIM = 64
B = 2
TGT = 1024
SRC = 2048
N_CORES = 8
HPC = 4  # heads per core
DPC = HPC * HEAD_DIM  # 256 projected dims per core
F32 = mybir.dt.float32
BF16 = mybir.dt.bfloat16
NPBF16 = ml_dtypes.bfloat16

TQ = 512  # t-chunk for attention units
S_TILES = SRC // P  # 16
KT = H_DIM // P  # 8 contraction tiles for projections
DT = DPC // P  # 2 d-tiles per core
NQ = TGT // TQ  # 2 t-chunks
NKC = SRC // TQ  # 4 n-chunks for k proj
VG = 4  # m-tile groups for v proj (4 s-tiles each)
NG = S_TILES // 2  # 8 attention groups per unit

# window -> group -> list of (owner_unit, j, m_lo, m_hi) PV drains
PV_TABLE = {
    1: {0: [(0, 0, 0, 7)], 1: [(0, 0, 8, 15)], 2: [(0, 1, 0, 7)],
        3: [(0, 1, 8, 15)], 4: [(1, 0, 0, 3)], 5: [(1, 0, 4, 7)],
        6: [(1, 0, 8, 11)], 7: [(1, 0, 12, 13)]},
    2: {0: [(1, 0, 14, 15), (1, 1, 0, 5)], 1: [(1, 1, 6, 11)],
        2: [(1, 1, 12, 15)], 3: [(2, 0, 0, 3)], 4: [(2, 0, 4, 7)],
        5: [(2, 0, 8, 9)], 6: [(2, 0, 10, 11)], 7: [(2, 0, 12, 13)]},
    3: {0: [(2, 0, 14, 15), (2, 1, 0, 5)], 1: [(2, 1, 6, 11)],
        2: [(2, 1, 12, 15)], 3: [(3, 0, 0, 3), (3, 1, 0, 1)],
        4: [(3, 0, 4, 7), (3, 1, 2, 5)], 5: [(3, 0, 8, 9), (3, 1, 6, 9)],
        6: [(3, 0, 10, 11), (3, 1, 10, 11)],
        7: [(3, 0, 12, 13), (3, 1, 12, 13)]},
}
# window -> group -> (owner_unit, j) norms (emitted before that group's PVs)
NORM_TABLE = {
    1: {2: [(0, 0)], 4: [(0, 1)]},
    2: {1: [(1, 0)], 3: [(1, 1)]},
    3: {1: [(2, 0)], 3: [(2, 1)]},
}

_prog_cache: dict = {}


def _emit(tc: tile.TileContext, outs, ins):
    nc = tc.nc
    xq, xk, xv, eb, wq, wk, wv, wo = ins
    (outT,) = outs
    Exp = mybir.ActivationFunctionType.Exp
    Copy = mybir.ActivationFunctionType.Copy

    with ExitStack() as ctx:
        const = ctx.enter_context(tc.tile_pool(name="const", bufs=1))
        xqpool = ctx.enter_context(tc.tile_pool(name="xqin", bufs=1))
        xkpool = ctx.enter_context(tc.tile_pool(name="xkin", bufs=2))
        xvpool = ctx.enter_context(tc.tile_pool(name="xvin", bufs=3))
        # peak eb in-flight is 10 tiles; 14 bufs leave a 2-group cushion so
        # a DVE stall can't sem-gate an eb DMA (one gated dma_start blocks
        # all 16 DMA queues behind it)
        ebpool = ctx.enter_context(tc.tile_pool(name="ebin", bufs=14))
        pgpool = ctx.enter_context(tc.tile_pool(name="pg", bufs=2))
        pppool = ctx.enter_context(tc.tile_pool(name="pp", bufs=2))
        rcpool = ctx.enter_context(tc.tile_pool(name="rcp", bufs=2))
        outp = ctx.enter_context(tc.tile_pool(name="outsb", bufs=3))
        psA = ctx.enter_context(tc.tile_pool(name="psA", bufs=2, space="PSUM"))
        psL = ctx.enter_context(tc.tile_pool(name="psL", bufs=1, space="PSUM"))
        psV = ctx.enter_context(tc.tile_pool(name="psV", bufs=1, space="PSUM"))

        # ---- persistent SBUF tensors ----
        wq_sb = const.tile([P, KT, DPC], BF16)  # [e_part, e_tile, d]
        wk_sb = const.tile([P, KT, DPC], BF16)
        wv_sb = const.tile([P, KT, DPC], BF16)
        wo_sb = const.tile([P, DT, H_DIM], BF16)  # [hd_part, hd_tile, e_out]
        q_sb = const.tile([P, DT, TGT], BF16)  # qT
        k_sb = const.tile([P, DT, SRC], BF16)  # kT
        # v plus 64 ones-columns, per (s_tile, head): [.., 0:64]=v, [.., 64:128]=1
        v_sb = const.tile([P, S_TILES, HPC, P], BF16)
        attn_sb = const.tile([P, DT, TGT], BF16)  # attnT, normalized

        nc.gpsimd.memset(v_sb[:, :, :, HEAD_DIM:P], 1.0)

        # ---- x chunk tiles (one DMA batch per chunk, deep prefetch) ----
        def x_chunk(pool, x_dram, n, nm):
            xt = pool.tile([P, KT, TQ], BF16, name=nm)
            for k in range(KT):
                nc.sync.dma_start(xt[:, k, :], x_dram[k, n])
            return xt

        # ---- q/k projections: psum[d_tile] += wT_tile.T @ xT_tile ----
        def proj_chunk(xt, w_sb, dst_sb, n, slots):
            pss = [slots() for _ in range(DT)]
            for k in range(KT):
                for m in range(DT):
                    nc.tensor.matmul(
                        pss[m][:],
                        lhsT=w_sb[:, k, m * P:(m + 1) * P],
                        rhs=xt[:, k, :],
                        start=(k == 0),
                        stop=(k == KT - 1),
                    )
            for m in range(DT):
                nc.vector.tensor_copy(
                    dst_sb[:, m, n * TQ:(n + 1) * TQ], pss[m][:])

        # ---- v projection group: 4 s-tiles, all heads ----
        def proj_v_group(xvt, mg, slots):
            for ml in range(VG):
                m = mg * VG + ml
                ps = slots()[:, :DPC]
                for k in range(KT):
                    nc.tensor.matmul(
                        ps,
                        lhsT=xvt[:, k, ml * P:(ml + 1) * P],
                        rhs=wv_sb[:, k, :],
                        start=(k == 0),
                        stop=(k == KT - 1),
                    )
                nc.vector.tensor_copy(
                    v_sb[:, m, :, 0:HEAD_DIM],
                    ps.rearrange("p (h d) -> p h d", d=HEAD_DIM),
                )

        # psum slot cyclers.  Preamble rotates through psA + the not-yet-
        # used psV slots so chunk copies never stall; in-window projs and
        # the out-projection use psA only.
        class Slots:
            def __init__(self, seq):
                self.seq = seq
                self.i = 0

            def __call__(self):
                pool, tag = self.seq[self.i % len(self.seq)]
                self.i += 1
                t = pool.tile([P, TQ], F32, name=f"ps_{tag}", tag=tag)
                return t

        pre_slots = Slots([(psA, "mm"), (psV, "pv0"), (psV, "pv1"), (psA, "mm")])
        mm_slots = Slots([(psA, "mm")])

        # ---- attention machinery ----
        units = [(0, 0), (1, 0), (0, 1), (1, 1)]  # (pair, tci) per unit idx
        ustate = [
            {"pair": p, "tci": t, "pvs": [None, None], "pp": None}
            for (p, t) in units
        ]

        def get_pp(ui):
            if ustate[ui]["pp"] is None:
                ustate[ui]["pp"] = pppool.tile(
                    [P, 2, S_TILES, TQ], BF16, name="pp", tag="pp")
            return ustate[ui]["pp"]

        ebtiles = {}

        def emit_eb(ui, g, j):
            pair, tci = units[ui]
            ebt = ebpool.tile([P, 2 * TQ], BF16, name="eb", tag="eb")
            nc.sync.dma_start(ebt[:], eb[2 * pair + j, tci, g])
            ebtiles[(ui, g, j)] = ebt

        def emit_pv(ui, j, m_lo, m_hi):
            st = ustate[ui]
            pp = get_pp(ui)
            if st["pvs"][j] is None:
                st["pvs"][j] = psV.tile(
                    [P, TQ], F32, name=f"pv{j}", tag=f"pv{j}")
            h = 2 * st["pair"] + j
            for m in range(m_lo, m_hi + 1):
                nc.tensor.matmul(
                    st["pvs"][j][:],
                    lhsT=v_sb[:, m, h, :],
                    rhs=pp[:, j, m, :],
                    start=(m == 0),
                    stop=(m == S_TILES - 1),
                )

        def norm_j(ui, j):
            """1/den on DVE (fast Newton-Raphson approx), one DVE mul -> attn.
            The exact `reciprocal` is 8 cyc/elem (~4us per norm) and stalls
            the in-order DVE queue behind it."""
            st = ustate[ui]
            pair, tci = st["pair"], st["tci"]
            t_sl = slice(tci * TQ, (tci + 1) * TQ)
            p0 = j * HEAD_DIM
            rc = rcpool.tile([P, TQ], F32, name="rc", tag="rc")
            # the custom-DVE approx mishandles partition-base offsets, so
            # run it over all 128 partitions; rows 0-63 (1/numerator) are
            # junk and never read
            nc.vector.reciprocal_approx_fast(rc[:], st["pvs"][j][:])
            nc.vector.tensor_mul(
                attn_sb[p0:p0 + HEAD_DIM, pair, t_sl],
                st["pvs"][j][0:HEAD_DIM, :],
                rc[HEAD_DIM:P, :],
            )
            st["pvs"][j] = None

        def attn_qk_group(ui, g):
            """QK for group g of unit ui + exp + eb-multiply."""
            st = ustate[ui]
            pair, tci = st["pair"], st["tci"]
            pp = get_pp(ui)
            ms = 2 * g
            t_sl = slice(tci * TQ, (tci + 1) * TQ)
            plss = []
            for j in range(2):
                tag = (2 * g + j) % 2
                plss.append(psL.tile([P, 2, TQ], F32, name=f"lg{tag}",
                                     tag=f"lg{tag}"))
            # j-adjacent issue: the two K=64 matmuls run in disjoint PE
            # row halves
            for mi in range(2):
                for j in range(2):
                    p0 = j * HEAD_DIM
                    nc.tensor.matmul(
                        plss[j][:, mi, :],
                        lhsT=k_sb[p0:p0 + HEAD_DIM, pair,
                                  (ms + mi) * P:(ms + mi + 1) * P],
                        rhs=q_sb[p0:p0 + HEAD_DIM, pair, t_sl],
                        start=True,
                        stop=True,
                    )
            for j in range(2):
                tag = (2 * g + j) % 2
                pg = pgpool.tile([P, 2, TQ], BF16, name=f"pg{tag}",
                                 tag=f"pg{tag}")
                nc.scalar.activation(pg[:], plss[j][:], Exp)
                ebt = ebtiles.pop((ui, g, j))
                nc.vector.tensor_mul(
                    pp[:, j, ms:ms + 2, :],
                    pg[:],
                    ebt.rearrange("p (m t) -> p m t", t=TQ),
                )

        # ---- out projection (partial; host sums head groups) ----
        def outproj_pair(tci, mo0, copy_engine):
            for mo in (mo0, mo0 + 1):
                ps = mm_slots()
                for kt in range(DT):
                    nc.tensor.matmul(
                        ps[:],
                        lhsT=wo_sb[:, kt, mo * P:(mo + 1) * P],
                        rhs=attn_sb[:, kt, tci * TQ:(tci + 1) * TQ],
                        start=(kt == 0),
                        stop=(kt == DT - 1),
                    )
                ot = outp.tile([P, TQ], BF16, name="ot")
                if copy_engine == "vector":
                    nc.vector.tensor_copy(ot[:], ps[:])
                else:
                    nc.scalar.activation(ot[:], ps[:], Copy)
                nc.sync.dma_start(outT[mo, tci], ot[:])

        # ================= emission schedule =================
        # The attention stream (QK -> exp -> eb-mul) is the critical chain:
        # the ACT exp ops (1.3us each, 2 per group) pace the whole kernel.
        # So: minimal preamble (q-t0 + k chunk 0 only -> first QK at ~13us),
        # then every other projection / out-projection chunk rides in the
        # PE slack after its window-group's QK.
        with nc.named_scope("pre"):
            nc.sync.dma_start(wq_sb[:], wq)
            xqt0 = x_chunk(xqpool, xq, 0, "xqt")
            nc.sync.dma_start(wk_sb[:], wk)
            xkt = [x_chunk(xkpool, xk, 0, "xkt")]
            proj_chunk(xqt0, wq_sb, q_sb, 0, pre_slots)
            xkt.append(x_chunk(xkpool, xk, 1, "xkt"))
            nc.sync.dma_start(wv_sb[:], wv)
            nc.sync.dma_start(wo_sb[:], wo)
            proj_chunk(xkt[0], wk_sb, k_sb, 0, pre_slots)
            for g in range(4):
                emit_eb(0, g, 0)
                emit_eb(0, g, 1)

        # aux work emitted AFTER (window, group)'s QK so a straggling input
        # DMA stalls the PE only where the exp stream covers it
        xvt, xqt1 = [], []

        def aux(w, g):
            if w == 0:
                if g == 0:
                    xvt.append(x_chunk(xvpool, xv, 0, "xvt"))
                    xvt.append(x_chunk(xvpool, xv, 1, "xvt"))
                elif g == 1:
                    proj_chunk(xkt[1], wk_sb, k_sb, 1, mm_slots)
                    xkt.append(x_chunk(xkpool, xk, 2, "xkt"))
                elif g == 2:
                    proj_v_group(xvt[0], 0, mm_slots)
                    xvt.append(x_chunk(xvpool, xv, 2, "xvt"))
                elif g == 3:
                    proj_chunk(xkt[2], wk_sb, k_sb, 2, mm_slots)
                    xkt.append(x_chunk(xkpool, xk, 3, "xkt"))
                elif g == 4:
                    proj_v_group(xvt[1], 1, mm_slots)
                    xvt.append(x_chunk(xvpool, xv, 3, "xvt"))
                elif g == 5:
                    proj_chunk(xkt[3], wk_sb, k_sb, 3, mm_slots)
                elif g == 6:
                    proj_v_group(xvt[2], 2, mm_slots)
                    xqt1.append(x_chunk(xqpool, xq, 1, "xqt"))
            elif w == 1:
                if g == 0:
                    proj_v_group(xvt[3], 3, mm_slots)
                elif g == 2:
                    proj_chunk(xqt1[0], wq_sb, q_sb, 1, mm_slots)
            elif w == 2 and g >= 4:
                outproj_pair(0, 2 * (g - 4), "vector")

        for w in range(4):
            with nc.named_scope(f"u{w}"):
                for g in range(NG):
                    # advance the eb stream cursor (4 groups ahead)
                    gg = 8 * w + g + 4
                    if gg < 32:
                        emit_eb(gg // 8, gg % 8, 0)
                        emit_eb(gg // 8, gg % 8, 1)
                    for (ui, j) in NORM_TABLE.get(w, {}).get(g, []):
                        norm_j(ui, j)
                    for (ui, j, mlo, mhi) in PV_TABLE.get(w, {}).get(g, []):
                        emit_pv(ui, j, mlo, mhi)
                    attn_qk_group(w, g)
                    aux(w, g)

        with nc.named_scope("tail"):
            emit_pv(3, 0, 14, 15)
            emit_pv(3, 1, 14, 15)
            norm_j(3, 0)
            norm_j(3, 1)
            for mo0 in range(0, H_DIM // P, 2):
                outproj_pair(1, mo0, "scalar")


def _build_program():
    key = ("prog", "bf16_v6")
    if key in _prog_cache:
        return _prog_cache[key]
    nc = bacc.Bacc("TRN2", target_bir_lowering=False, debug=False,
                   num_devices=N_CORES)
    ins = [
        nc.dram_tensor("xq", [KT, NQ, P, TQ], BF16, kind="ExternalInput").ap(),
        nc.dram_tensor("xk", [KT, NKC, P, TQ], BF16, kind="ExternalInput").ap(),
        nc.dram_tensor("xv", [KT, VG, P, TQ], BF16, kind="ExternalInput").ap(),
        nc.dram_tensor("eb", [HPC, NQ, NG, P, 2 * TQ], BF16,
                       kind="ExternalInput").ap(),
        nc.dram_tensor("wq", [P, KT, DPC], BF16, kind="ExternalInput").ap(),
        nc.dram_tensor("wk", [P, KT, DPC], BF16, kind="ExternalInput").ap(),
        nc.dram_tensor("wv", [P, KT, DPC], BF16, kind="ExternalInput").ap(),
        nc.dram_tensor("wo", [P, DT, H_DIM], BF16, kind="ExternalInput").ap(),
    ]
    outs = [nc.dram_tensor("outT", [H_DIM // P, NQ, P, TQ], BF16,
                           kind="ExternalOutput").ap()]
    with tile.TileContext(nc) as tc:
        _emit(tc, outs, ins)
    nc.compile()
    _prog_cache[key] = nc
    return nc


def _tile_x(xT):
    """[E, L] -> [KT, L//TQ, P, TQ] contiguous tiles."""
    E, L = xT.shape
    return np.ascontiguousarray(
        xT.reshape(KT, P, L // TQ, TQ).transpose(0, 2, 1, 3)).astype(NPBF16)


def _host_prep(query, key, value, attn_bias, attention_mask,
               Wq, bq, Wk, bk, Wv, bv, Wo, bo):
    """Build the 8 per-core input maps (all bf16, pre-tiled)."""
    f = np.float32
    query = np.asarray(query, f)
    key = np.asarray(key, f)
    value = np.asarray(value, f)
    attn_bias = np.asarray(attn_bias, f)
    mask = np.asarray(attention_mask)
    Wq = np.asarray(Wq, f); bq = np.asarray(bq, f)
    Wk = np.asarray(Wk, f)
    Wv = np.asarray(Wv, f)
    Wo = np.asarray(Wo, f)

    scale = f(1.0 / np.sqrt(HEAD_DIM))
    # c[b, s, h] = scale * (bq_h . k_h(s)) with k = key @ Wk^T (no bk —
    # bk cancels in softmax). U[e, h] = sum_{d in head h} Wk[d, e] bq[d].
    U = (Wk * (bq * scale)[:, None]).reshape(N_HEADS, HEAD_DIM, H_DIM)
    U = U.sum(axis=1)  # [H, E]
    c = np.einsum("bse,he->bsh", key, U)  # [B, S, H]

    # exp'd masked bias: eb[b,h,s,t] = exp(bias[b,h,t,s] + c[b,s,h]); 0 masked
    ebias = np.exp(attn_bias.transpose(0, 1, 3, 2)
                   + c.transpose(0, 2, 1)[:, :, :, None])
    maskT = mask.transpose(0, 2, 1)[:, None, :, :]  # [B, 1, S, T]
    ebias = np.where(maskT, f(0.0), ebias)
    # tile: [B, H, S, T] -> [B, H, NQ, NG(g), P, (mm, t)]
    # s = g*256 + mm*128 + p ; t = tci*TQ + tt
    ebias = ebias.reshape(B, N_HEADS, NG, 2, P, NQ, TQ)
    # axes: [b, h, g, mm, p, tci, tt] -> [b, h, tci, g, p, mm, tt]
    ebias = np.ascontiguousarray(
        ebias.transpose(0, 1, 5, 2, 4, 3, 6)).reshape(
        B, N_HEADS, NQ, NG, P, 2 * TQ).astype(NPBF16)

    xqT = [_tile_x(query[b].T) for b in range(B)]
    xkT = [_tile_x(key[b].T) for b in range(B)]
    xvT = [_tile_x(value[b].T) for b in range(B)]

    def tile_w(wT):  # [E=1024, D=256] -> [128, 8, 256]
        return np.ascontiguousarray(
            wT.reshape(KT, P, DPC).transpose(1, 0, 2)).astype(NPBF16)

    in_maps = []
    for cc in range(N_CORES):
        b, g = divmod(cc, N_CORES // B)
        hs = g * HPC
        he = hs + HPC
        ds_, de = hs * HEAD_DIM, he * HEAD_DIM
        in_maps.append({
            "xq": xqT[b],
            "xk": xkT[b],
            "xv": xvT[b],
            "eb": np.ascontiguousarray(ebias[b, hs:he]),
            "wq": tile_w((Wq[ds_:de] * scale).T),
            "wk": tile_w(Wk[ds_:de].T),
            "wv": tile_w(Wv[ds_:de].T),
            "wo": np.ascontiguousarray(
                Wo[:, ds_:de].T.reshape(DT, P, H_DIM).transpose(1, 0, 2)
            ).astype(NPBF16),
        })
    return in_maps


def _assemble(results, Wo, bv, bo):
    Wo = np.asarray(Wo, np.float64)
    bv = np.asarray(bv, np.float64)
    bo = np.asarray(bo, np.float64)
    bconst = Wo @ bv + bo  # [H_DIM]
    G = N_CORES // B
    out = np.empty((B, TGT, H_DIM), np.float32)
    for b in range(B):
        acc = np.zeros((H_DIM, TGT), np.float64)
        for g in range(G):
            blk = np.asarray(results[b * G + g]["outT"], np.float32)
            acc += blk.transpose(0, 2, 1, 3).reshape(H_DIM, TGT)
        out[b] = (acc.T + bconst[None, :]).astype(np.float32)
    return out


def kernel(**inputs):
    in_maps = _host_prep(**inputs)
    nc = _build_program()
    res = run_bass_kernel_spmd(nc, in_maps, core_ids=list(range(N_CORES)))
    return _assemble(res.results, inputs["Wo"], inputs["bv"], inputs["bo"])
